# revision 35
# baseline (speedup 1.0000x reference)
"""Causal self-attention (q/k-swapped variant) Bass kernel for Trainium2.

Problem: B=2, T=2048, C=768, H=12, hs=64.
    k = x@Wk+bk ; q = x@Wq+bq ; v = x@Wv+bv          (per-head split)
    att[b,h,i,j] = (k[b,i,h,:] . q[b,j,h,:]) / 8     (note: k rows, q cols)
    att = softmax(causal-mask(att), axis=j)
    y = (att @ v) @ Wo + bo

Sharding: 8 cores = 2 batches x 4 head-groups (3 heads each).
Each core computes its 3 heads fully (QKV proj -> attention -> partial
output projection); host sums the 4 partial outputs per batch and adds bo.

Score math runs in "transposed score" space: score tiles have j (softmax
axis) on partitions and i on the free dim, so the PV matmul needs no
transposes, and the softmax denominator falls out of the PV matmul via an
appended ones-column on V.

Schedule: 4 i-windows of 512 columns. Per window, a (h0,h1) pair unit runs
both heads' K=64 score matmuls on disjoint PE row groups (h0 rows 0:63,
h1 rows 64:127) so they execute concurrently, with one merged exp per
j-block; then an h2 unit runs h2's scores on rows 64:127 packed against
output-projection matmuls (K=64 on rows 0:63) for the previous window.
"""

import os
import sys

sys.path.insert(0, "/opt/trn_rl_repo")

import numpy as np

T = 2048
C = 768
HS = 64
HPC = 3          # heads per core
NCH = C // 128   # 6 contraction chunks
TB = T // 128    # 16 j blocks
NW = 4           # i-windows
WW = T // NW     # window width = 512
NCORES = 8
MM_DTYPE = os.environ.get("KERNEL_MM_DTYPE", "fp16")  # fp16 | bf16 | fp32
WARMUP = int(os.environ.get("KERNEL_WARMUP", "24"))
PV_DELAY = int(os.environ.get("KERNEL_PV_DELAY", "4"))

_cache = {}


def _emit(ctx, tc):
    import concourse.bass as bass
    import concourse.tile as tile  # noqa: F401
    from concourse import mybir
    from concourse.bass import ts
    from concourse.masks import make_upper_triangular

    f32 = mybir.dt.float32
    mmd = {"fp16": mybir.dt.float16, "bf16": mybir.dt.bfloat16,
           "fp32": f32}[MM_DTYPE]  # matmul-input dtype
    nc = tc.nc

    xT = nc.dram_tensor("xT", (C, T), mmd, kind="ExternalInput").ap()
    wqk = nc.dram_tensor("wqk", (128, 3 * NCH * 128), mmd, kind="ExternalInput").ap()
    wv = nc.dram_tensor("wv", (128, NCH * 192), mmd, kind="ExternalInput").ap()
    wo = nc.dram_tensor("wo", (64, 3 * C), mmd, kind="ExternalInput").ap()
    bqk = nc.dram_tensor("bqk", (128, 3), f32, kind="ExternalInput").ap()
    bv = nc.dram_tensor("bv", (1, 192), f32, kind="ExternalInput").ap()
    y = nc.dram_tensor("y", (C, T), mmd, kind="ExternalOutput").ap()  # transposed

    consts = ctx.enter_context(tc.tile_pool(name="consts", bufs=1))

    # ---- load inputs, split into many small DMAs (each lands on one DMA
    # engine; ~20 GB/s per engine, 16 engines) ordered so the first window's
    # inputs (wqk g0/g1, xT cols 0:512, wv) complete first ----
    wqkr = wqk.rearrange("p (g k m) -> p g k m", g=3, k=NCH)
    bqk_sb = consts.tile([128, 3], f32)       # per-partition bias per QK group
    nc.sync.dma_start(bqk_sb[:], bqk)
    bvb_sb = consts.tile([128, 192], f32)     # bv broadcast across partitions
    nc.sync.dma_start(bvb_sb[:], bv.to_broadcast((128, 192)))
    wqk_sb = consts.tile([128, 3, NCH, 128], mmd)
    xT_sb = consts.tile([128, NCH, T], mmd)
    wv_sb = consts.tile([128, NCH, 192], mmd)
    # per-g wqk keeps 1.5KB contiguous lines per partition; xT column blocks
    # have 1KB lines; wv split by partition halves keeps its 2.3KB lines
    # it0+it1 issued back-to-back with the qk weights: exactly 16 DMAs to
    # saturate all 16 engines from t0; later-needed tensors follow as
    # engines free up (wv ~13us, g2 ~18us, it2 ~19us, wo ~27us)
    nc.sync.dma_start(wqk_sb[:, 0], wqkr[:, 0])
    nc.sync.dma_start(wqk_sb[:, 1], wqkr[:, 1])
    for it in range(2):
        for k in range(NCH):
            nc.sync.dma_start(xT_sb[:, k, ts(it, 512)],
                              xT[k * 128:(k + 1) * 128, ts(it, 512)])
    nc.sync.dma_start(wv_sb[0:64], wv[0:64].rearrange("p (k m) -> p k m", k=NCH))
    nc.sync.dma_start(wv_sb[64:128], wv[64:128].rearrange("p (k m) -> p k m", k=NCH))
    nc.sync.dma_start(wqk_sb[:, 2], wqkr[:, 2])
    for k in range(NCH):
        nc.sync.dma_start(xT_sb[:, k, ts(2, 512)],
                          xT[k * 128:(k + 1) * 128, ts(2, 512)])
    wo_sb = consts.tile([64, 3, C], mmd)
    nc.sync.dma_start(wo_sb[:], wo.rearrange("p (h c) -> p h c", h=3))
    for k in range(NCH):
        nc.sync.dma_start(xT_sb[:, k, ts(3, 512)],
                          xT[k * 128:(k + 1) * 128, ts(3, 512)])

    scratch = consts.tile([128, 512], mmd)
    nc.vector.memset(scratch[:], 0.0)
    trimask = consts.tile([128, 1, 128], mmd)
    make_upper_triangular(nc, trimask[:, 0, :], val=1.0, diag=True)

    V_aug = consts.tile([128, TB, HPC * 65], mmd)
    for h in range(HPC):
        nc.vector.memset(V_aug[:, :, h * 65 + 64:h * 65 + 65], 1.0)

    QK_sb = consts.tile([128, 3, T], mmd)     # g0=Q(h0,h1) g1=K(h0,h1) g2=[K(h2)|Q(h2)]
    KT2_sb = consts.tile([128, T], mmd)       # K(h2) shifted to partitions 64:127
    AT_sb = consts.tile([64, HPC, T], mmd)    # normalized attn output, transposed

    # ---- pools ----
    # PSUM banks: psP 2 + psS (2 bufs x [128,2,512] = 2 banks each) 4 + psO 2 = 8
    psP = ctx.enter_context(tc.tile_pool(name="psP", bufs=2, space="PSUM"))
    psS = ctx.enter_context(tc.tile_pool(name="psS", bufs=2, space="PSUM"))
    psO = ctx.enter_context(tc.tile_pool(name="psO", bufs=1, space="PSUM"))
    sbE = ctx.enter_context(tc.tile_pool(name="E", bufs=7))
    sbATn = ctx.enter_context(tc.tile_pool(name="ATn", bufs=2))
    sbRZ = ctx.enter_context(tc.tile_pool(name="RZ", bufs=2))
    sbY = ctx.enter_context(tc.tile_pool(name="Y", bufs=4))

    def dummy(n=512):
        warm = psP.tile([128, 512], f32, tag="p")
        nc.tensor.matmul(warm[:, 0:n], lhsT=scratch[:, 0:128], rhs=scratch[:, 0:n],
                         start=True, stop=True, skip_group_check=True)

    # PE warm-up: keeps the HAM at full clock while inputs stream in, and
    # covers the DMA latency of the first projection inputs. Also pre-load
    # the exp table (~2.7us one-time) early.
    for _ in range(WARMUP):
        dummy()
    edum = sbE.tile([128, 2, 512], mmd)
    nc.scalar.activation(edum[:, 0, :], scratch[:],
                         mybir.ActivationFunctionType.Exp, scale=0.125)

    def qk_group(g, it):
        ps = psP.tile([128, 512], f32, tag="p")
        for k in range(NCH):
            nc.tensor.matmul(ps[:], lhsT=wqk_sb[:, g, k, :],
                             rhs=xT_sb[:, k, ts(it, 512)],
                             start=(k == 0), stop=(k == NCH - 1))
        nc.vector.tensor_add(QK_sb[:, g, ts(it, 512)], ps[:],
                             bqk_sb[:, g:g + 1].to_broadcast((128, 512)))

    def v_group(tb):
        ps = psP.tile([128, 512], f32, tag="p")
        for k in range(NCH):
            nc.tensor.matmul(ps[:, 0:192], lhsT=xT_sb[:, k, ts(tb, 128)],
                             rhs=wv_sb[:, k, :],
                             start=(k == 0), stop=(k == NCH - 1))
        nc.any.tensor_add(
            V_aug[:, tb].rearrange("p (h m) -> p h m", h=HPC)[:, :, 0:64],
            ps[:, 0:192].rearrange("p (h m) -> p h m", h=HPC),
            bvb_sb[:].rearrange("p (h m) -> p h m", h=HPC))

    def kt2_shift(it):
        # K(h2) lives at partitions 0:63 of group 2; move to 64:127 so h2
        # score matmuls occupy PE rows 64:127 (QT2 is already there).
        nc.sync.dma_start(KT2_sb[64:128, ts(it, 512)], QK_sb[0:64, 2, ts(it, 512)])

    def oproj_group(w, cb):
        ps = psP.tile([128, 512], f32, tag="p")
        for hh in range(HPC):
            nc.tensor.matmul(ps[:], lhsT=wo_sb[:, hh, ts(cb, 128)],
                             rhs=AT_sb[:, hh, ts(w, WW)],
                             start=(hh == 0), stop=(hh == HPC - 1))
        ysb = sbY.tile([128, 512], mmd)
        if w == NW - 1:
            # tail: halve the drain latency by splitting across both
            # PSUM-capable engines so the final y DMAs issue sooner
            nc.vector.tensor_copy(ysb[:, 0:256], ps[:, 0:256])
            nc.scalar.copy(ysb[:, 256:512], ps[:, 256:512])
        else:
            nc.any.tensor_copy(ysb[:], ps[:])
        # y goes out on the gpsimd queue: the sync queue's FIFO otherwise
        # blocks later kt2/z-chain DMAs behind y writes that wait on oproj.
        # The last window's writes are on the critical tail: split into
        # 64KB halves round-robined over three issue queues so both the
        # ~0.6us per-issue cost and the per-engine transfer parallelize.
        if w == NW - 1:
            qs = [nc.gpsimd, nc.sync, nc.scalar]
            qa, qb = qs[cb % 3], qs[(cb + 1) % 3]
            qa.dma_start(y[cb * 128:cb * 128 + 64, ts(w, WW)], ysb[0:64])
            qb.dma_start(y[cb * 128 + 64:cb * 128 + 128, ts(w, WW)],
                         ysb[64:128])
        else:
            nc.gpsimd.dma_start(y[cb * 128:(cb + 1) * 128, ts(w, WW)], ysb[:])

    from collections import deque
    # pre_fillers carry forward-data hazards (Tile deps are emission-order
    # based). Barriers: before each unit, force-pop until its inputs are
    # emitted. op_fillers (output projection) only read already-written data.
    pre_fillers = deque()
    barrier = {}  # (w, kind) -> count of pre_fillers that must be emitted

    def _add(fns):
        for fn in fns:
            pre_fillers.append(fn)

    # emission requirements per unit (in pop order); kt2 shifts are emitted
    # a window early so their DMA isn't on any unit's critical path
    _add([lambda: qk_group(2, 0), lambda: kt2_shift(0)])
    barrier[(0, 'h2')] = len(pre_fillers)
    for w in range(1, NW):
        _add([lambda w=w: qk_group(0, w), lambda w=w: qk_group(1, w),
              lambda w=w: qk_group(2, w), lambda w=w: kt2_shift(w)])
        barrier[(w, 'pair')] = len(pre_fillers)
        # window w's V blocks pop inside its own pair unit (PV trails the
        # scores by PV_DELAY steps, so v(tb) lands in time as the step-tb pop)
        _add([(lambda tb=tb: v_group(tb)) for tb in range(4 * w, 4 * w + 4)])
    _popped = [0]
    op_fillers = deque()

    def pop_filler(allow_dummy=True, allow_op=True):
        if pre_fillers:
            pre_fillers.popleft()()
            _popped[0] += 1
            return
        if allow_op and op_fillers:
            op_fillers.popleft()()
            return
        if allow_dummy:
            dummy()

    def force_to(key):
        need = barrier.get(key, 0) - _popped[0]
        for _ in range(max(0, need)):
            pre_fillers.popleft()()
            _popped[0] += 1

    # pre-phase: W0 pair needs Q,K cols 0:512 for h0/h1; its V blocks pop
    # inside the unit
    qk_group(0, 0)
    qk_group(1, 0)
    pre_fillers.extendleft([(lambda tb=tb: v_group(tb))
                            for tb in reversed(range(4))])
    for k in barrier:
        barrier[k] += 4

    def normalize(h, w, Onum):
        """Copy Onum out of PSUM, compute 1/Z (row 64) via a DMA-reshape
        round trip for a parallel reciprocal, broadcast over 64 partitions,
        and write normalized AT columns."""
        c0 = w * WW
        ATn = sbATn.tile([65, WW], f32)
        nc.vector.tensor_copy(ATn[:], Onum[:])
        # z-chain DMAs ride the sync queue, which after the input loads only
        # carries kt2 shifts (y output rides gpsimd instead)
        z16 = sbRZ.tile([128, WW // 128], f32, tag="z16")
        nc.sync.dma_start(z16[:], ATn[64:65, :])
        r16 = sbRZ.tile([128, WW // 128], f32, tag="r16")
        nc.vector.reciprocal(r16[:], z16[:])
        rz1 = sbRZ.tile([1, WW], f32, tag="rz1")
        nc.sync.dma_start(rz1[:], r16[:])
        rzb = sbRZ.tile([64, WW], f32, tag="rzb")
        nc.gpsimd.partition_broadcast(rzb[:], rz1[:], channels=64)
        nc.vector.tensor_mul(AT_sb[:, h, c0:c0 + WW], ATn[0:64, :], rzb[:])

    heads = [
        (QK_sb[0:64, 0, :], QK_sb[0:64, 1, :]),        # h0: rows 0:63
        (QK_sb[64:128, 0, :], QK_sb[64:128, 1, :]),    # h1: rows 64:127
        (QK_sb[64:128, 2, :], KT2_sb[64:128, :]),      # h2: rows 64:127
    ]

    for w in range(NW):
        c0 = w * WW
        njb = (c0 + WW) // 128

        # ---- pair unit: h0 and h1 score matmuls run concurrently on
        # disjoint PE row groups; one merged exp per j-block ----
        force_to((w, 'pair'))
        Onum0 = psO.tile([65, WW], f32, tag="o0")
        Onum1 = psO.tile([65, WW], f32, tag="o1")
        pending = []

        def emit_pv_pair(jb, E, r):
            for h, On in ((0, Onum0), (1, Onum1)):
                nc.tensor.matmul(On[:, r:], lhsT=V_aug[:, jb, h * 65:(h + 1) * 65],
                                 rhs=E[:, h, r:], start=(jb == 0),
                                 stop=(jb == njb - 1), skip_group_check=True)

        for jb in range(njb):
            i0 = 128 * jb
            r = max(0, i0 - c0)  # causal clip inside window
            S = psS.tile([128, 2, WW], f32)
            for h in (0, 1):
                QT, KT = heads[h]
                nc.tensor.matmul(S[:, h, r:], lhsT=QT[:, ts(jb, 128)],
                                 rhs=KT[:, c0 + r:c0 + WW], start=True, stop=True)
            E = sbE.tile([128, 2, WW], mmd)
            nc.scalar.activation(E[:, :, r:], S[:, :, r:],
                                 mybir.ActivationFunctionType.Exp, scale=0.125)
            if r == i0 - c0 and i0 >= c0:  # diagonal block
                nc.vector.tensor_mul(E[:, :, r:r + 128], E[:, :, r:r + 128],
                                     trimask.to_broadcast((128, 2, 128)))
            # oproj groups are reserved for h2 units where they pack onto
            # PE rows 0:63 for free; here they'd serialize with h0's scores
            pop_filler(allow_op=False)
            pending.append((jb, E, r))
            if len(pending) > PV_DELAY:
                emit_pv_pair(*pending.pop(0))
        for item in pending:
            emit_pv_pair(*item)
        normalize(0, w, Onum0)
        normalize(1, w, Onum1)

        # ---- h2 unit: h2 scores on rows 64:127, packed against previous
        # window's output projection on rows 0:63 ----
        force_to((w, 'h2'))
        QT2, KT2 = heads[2]
        Onum2 = psO.tile([65, WW], f32, tag="o0")
        pending = []

        def emit_pv2(jb, Ep, r):
            nc.tensor.matmul(Onum2[:, r:], lhsT=V_aug[:, jb, 130:195],
                             rhs=Ep[:, r:], start=(jb == 0),
                             stop=(jb == njb - 1), skip_group_check=True)

        jb = 0
        while jb < njb:
            i0 = 128 * jb
            r = max(0, i0 - c0)
            S = psS.tile([128, 2, WW], f32)
            if jb + 1 < 4 * w:  # both jb and jb+1 fully below the diagonal:
                # two score matmuls share one exp call (ACT call overhead)
                nc.tensor.matmul(S[:, 0, :], lhsT=QT2[:, ts(jb, 128)],
                                 rhs=KT2[:, c0:c0 + WW], start=True, stop=True)
                nc.tensor.matmul(S[:, 1, :], lhsT=QT2[:, ts(jb + 1, 128)],
                                 rhs=KT2[:, c0:c0 + WW], start=True, stop=True)
                E = sbE.tile([128, 2, WW], mmd)
                nc.scalar.activation(E[:], S[:],
                                     mybir.ActivationFunctionType.Exp,
                                     scale=0.125)
                if op_fillers:  # packs onto rows 0:63 concurrent with ST2
                    op_fillers.popleft()()
                pop_filler(allow_dummy=False, allow_op=False)
                pending.append((jb, E[:, 0, :], 0))
                pending.append((jb + 1, E[:, 1, :], 0))
                jb += 2
            else:
                nc.tensor.matmul(S[:, 0, r:], lhsT=QT2[:, ts(jb, 128)],
                                 rhs=KT2[:, c0 + r:c0 + WW], start=True,
                                 stop=True)
                E = sbE.tile([128, 2, WW], mmd)
                nc.scalar.activation(E[:, 0, r:], S[:, 0, r:],
                                     mybir.ActivationFunctionType.Exp,
                                     scale=0.125)
                if i0 >= c0:
                    nc.vector.tensor_mul(E[:, 0, r:r + 128], E[:, 0, r:r + 128],
                                         trimask[:, 0, :])
                if op_fillers:  # packs onto rows 0:63 concurrent with ST2
                    op_fillers.popleft()()
                pop_filler(allow_dummy=False, allow_op=False)
                pending.append((jb, E[:, 0, :], r))
                jb += 1
            while len(pending) > PV_DELAY:
                emit_pv2(*pending.pop(0))
        for item in pending:
            emit_pv2(*item)
        normalize(2, w, Onum2)

        # window w fully normalized: its output projection becomes available
        for cb in range(NCH):
            op_fillers.append(lambda w=w, cb=cb: oproj_group(w, cb))

    # ---- tail: bridge the last z-chain's ~6us latency with dummies so the
    # PE clock stays at 2.4GHz for the final output projection. These use
    # the (now idle) psS banks - the psP ring is pinned by in-flight oproj
    # tiles whose drain copies haven't run yet. ----
    for _ in range(44):
        S = psS.tile([128, 2, WW], f32)  # noqa: F841 - tag-shares the S ring
        nc.tensor.matmul(S[:, 0, :], lhsT=scratch[:, 0:128],
                         rhs=scratch[:], start=True, stop=True,
                         skip_group_check=True)
    while pre_fillers or op_fillers:
        pop_filler(allow_dummy=False)
        if op_fillers:
            op_fillers.popleft()()


def _build():
    if "nc" in _cache:
        return _cache["nc"]
    from contextlib import ExitStack

    import concourse.tile as tile
    from concourse import bacc

    nc = bacc.Bacc("TRN2", target_bir_lowering=False, debug=False,
                   num_devices=NCORES)
    with tile.TileContext(nc) as tc:
        with ExitStack() as ctx:
            _emit(ctx, tc)
    nc.compile()
    _cache["nc"] = nc
    return nc


def _install_trace_hooks():
    """Make trace=True work in this container: shim the missing
    antenv.axon_hooks NTFF-profile hook (ctypes into libaxon_pjrt.so) and
    skip the S3 artifact upload."""
    import contextlib
    import ctypes
    import types

    import concourse.bass_utils as bu

    bu.upload_artifacts = lambda tmpdir: tmpdir
    try:
        from antenv.axon_hooks import get_axon_ntff_profile_hook  # noqa: F401
        return
    except ImportError:
        pass

    so_path = "/opt/axon/libaxon_pjrt.so"
    if not os.path.exists(so_path):
        return
    lib = ctypes.CDLL(so_path)
    if not hasattr(lib, "axon_start_nrt_profile"):
        return
    lib.axon_start_nrt_profile.argtypes = [
        ctypes.POINTER(ctypes.c_int64), ctypes.c_size_t,
    ]
    lib.axon_start_nrt_profile.restype = ctypes.c_int64
    lib.axon_stop_nrt_profile.argtypes = [ctypes.c_char_p]
    lib.axon_stop_nrt_profile.restype = ctypes.c_int64

    @contextlib.contextmanager
    def _hook(output_dir, device_ids):
        import jax
        jax.devices()
        if device_ids:
            ids = (ctypes.c_int64 * len(device_ids))(*device_ids)
            rc = lib.axon_start_nrt_profile(ids, len(device_ids))
        else:
            rc = lib.axon_start_nrt_profile(None, 0)
        if rc != 0:
            raise RuntimeError(f"axon_start_nrt_profile rc={rc}")
        try:
            yield
        finally:
            n = lib.axon_stop_nrt_profile(str(output_dir).encode())
            print(f"profile: {n} file(s) written to {output_dir}",
                  file=sys.stderr)

    state = {"h": _hook}
    mod = types.ModuleType("antenv.axon_hooks")
    mod.get_axon_ntff_profile_hook = lambda: state["h"]
    mod.set_axon_ntff_profile_hook = lambda h: state.__setitem__("h", h)
    import antenv
    antenv.axon_hooks = mod
    sys.modules["antenv.axon_hooks"] = mod


def kernel(**inputs):
    x = np.ascontiguousarray(np.asarray(inputs["x"], dtype=np.float32))
    Wq = np.asarray(inputs["Wq"], dtype=np.float32)
    Wk = np.asarray(inputs["Wk"], dtype=np.float32)
    Wv = np.asarray(inputs["Wv"], dtype=np.float32)
    Wo = np.asarray(inputs["Wo"], dtype=np.float32)
    bq = np.asarray(inputs["bq"], dtype=np.float32)
    bk = np.asarray(inputs["bk"], dtype=np.float32)
    bv = np.asarray(inputs["bv"], dtype=np.float32)
    bo = np.asarray(inputs["bo"], dtype=np.float32)

    from concourse import bass_utils

    nc = _build()

    if MM_DTYPE == "bf16":
        import ml_dtypes
        mmd_np = ml_dtypes.bfloat16
    elif MM_DTYPE == "fp16":
        mmd_np = np.float16
    else:
        mmd_np = np.float32

    B = x.shape[0]
    xTs = [np.ascontiguousarray(x[b].T.astype(mmd_np)) for b in range(B)]
    in_maps = []
    for core in range(NCORES):
        b, hg = core // 4, core % 4
        sl = slice(hg * 192, (hg + 1) * 192)
        wq_s, wk_s = Wq[:, sl], Wk[:, sl]
        g0 = wq_s[:, 0:128]
        g1 = wk_s[:, 0:128]
        g2 = np.concatenate([wk_s[:, 128:192], wq_s[:, 128:192]], axis=1)
        wqk_h = (np.stack([g0, g1, g2], 0)
                 .reshape(3, NCH, 128, 128).transpose(2, 0, 1, 3)
                 .reshape(128, 3 * NCH * 128))
        wv_h = (Wv[:, sl].reshape(NCH, 128, 192).transpose(1, 0, 2)
                .reshape(128, NCH * 192))
        wo_h = (Wo[sl, :].reshape(3, 64, C).transpose(1, 0, 2)
                .reshape(64, 3 * C))
        bqk_h = np.stack(
            [bq[sl][0:128], bk[sl][0:128],
             np.concatenate([bk[sl][128:192], bq[sl][128:192]])], axis=1
        )  # [128, 3]
        bv_h = bv[sl].reshape(1, 192)
        in_maps.append({
            "xT": xTs[b],
            "wqk": np.ascontiguousarray(wqk_h.astype(mmd_np)),
            "wv": np.ascontiguousarray(wv_h.astype(mmd_np)),
            "wo": np.ascontiguousarray(wo_h.astype(mmd_np)),
            "bqk": np.ascontiguousarray(bqk_h),
            "bv": np.ascontiguousarray(bv_h),
        })

    trace = bool(os.environ.get("KERNEL_TRACE"))
    if trace:
        _install_trace_hooks()
    res = bass_utils.run_bass_kernel_spmd(
        nc, in_maps, core_ids=list(range(NCORES)), trace=trace
    )
    _cache["last_results"] = res

    out = np.empty((B, T, C), dtype=np.float32)
    for b in range(B):
        acc = res.results[b * 4]["y"].astype(np.float32)
        for hg in range(1, 4):
            acc += res.results[b * 4 + hg]["y"].astype(np.float32)
        out[b] = acc.T + bo
    return out
